# revision 8
# baseline (speedup 1.0000x reference)
"""Trainium2 Bass kernel for nn_Coboundary: y[b,o,n] = sum_c theta[o,c] * sum_m D[n,m] x[b,c,m] + bias.

Strategy (memory-bound, D is 1 GiB fp32):
  - Host folds theta into x:  w[bo, m] = sum_c theta[o,c] x[b,c,m]  (bo = b*8+o, 16 rows).
  - Host quantizes D to fp8-e3m4 (4 mantissa bits; exact-data max-metric rel err
    ~1.1e-2 vs the 2e-2 gate) and pre-packs each core's slice D[c*2048:(c+1)*2048, :].T
    into the exact SBUF slab layout [n_slabs, 128, slab_mt, n_loc] so every DMA is a
    straight contiguous copy (32 KiB/partition descriptors).
  - Device (per core, mode fp8s): D tiles [128m, 128n] are the PE *stationary* operand
    (fast-weight-load ingests fp8 weights ~2/cycle/lane), w [128m, 16] is the moving
    operand -> psum[n, bo] accumulated over the 128 m-tiles, 16 regions spread over
    all 8 PSUM banks. Measured ~116 us/rep: PE-bound at ~133 cyc/tile (MM NX-dispatch
    floor + LDWEIGHTS fill), just above the ~91 us HBM floor (32 MiB/core at the
    measured ~378 GB/s/core). Beats D-moving (1 col/cycle -> ~127 us) and is 3.3x
    over the fp32r baseline (355-384 us).
  - Host re-assembles [2,8,16384] from the per-core outputs and adds bias.
"""

import sys
import numpy as np

for _p in ("/opt/trn_rl_repo", "/root/.axon_site/_ro/trn_rl_repo"):
    if _p not in sys.path:
        sys.path.append(_p)

N = 16384
M = 16384
B = 2
C_IN = 4
C_OUT = 8
BO = B * C_OUT  # 16
N_CORES = 8
N_LOC = N // N_CORES  # 2048

P = 128               # partition / contraction tile
NB = 512              # matmul moving free dim in D-moving modes (one PSUM bank fp32)
N_BLOCKS = N_LOC // NB          # 4
M_TILES = M // P                # 128
NT = N_LOC // P                 # 16 n-tiles (fp8s mode)

MODE = "fp8s"  # 'fp32r' | 'bf16' | 'fp8' | 'fp8mx' | 'fp8s'

_RUNNERS = {}


def _mode_cfg(mode):
    """-> (d_dt_name, w_dt_name, slab_mt, dma_chunks, stationary_d)"""
    return {
        "fp32r": ("float32r", "float32r", 8, 8, False),
        "bf16": ("bfloat16", "bfloat16", 8, 4, False),
        "fp8": ("float8e3", "float8e3", 16, 4, False),
        "fp8mx": ("float8e3", "bfloat16", 16, 4, False),
        "fp8s": ("float8e3", "bfloat16", 16, 4, True),
    }[mode]


# ---------------------------------------------------------------------------
# Walrus workaround: this compiler build allows only one sync-wait slot per
# instruction (CTRL and S3_LW templates alike), but Tile emits instructions
# carrying one wait per producer proc. Post-process the scheduled program and
# hoist surplus waits onto same-engine NoOps inserted immediately before the
# offending instruction (sequential waits are equivalent for monotonic sems).
# ---------------------------------------------------------------------------
def _split_multi_waits(nc):
    import concourse.mybir as mybir

    for f in nc.m.functions:
        for bb in f.blocks:
            out = []
            changed = False
            for inst in bb.instructions:
                si = getattr(inst, "sync_info", None)
                waits = list(si.on_wait) if si is not None and si.on_wait else []
                if len(waits) > 1:
                    changed = True
                    for w in waits[:-1]:
                        nop = mybir.InstNoOp(
                            name=nc.get_next_instruction_name(), ins=[], outs=[]
                        )
                        nop.engine = inst.engine
                        nop.sync_info = mybir.SyncInfo(on_wait=[w], on_update=[])
                        nc.register_instruction(nop, overwrite=True)
                        out.append(nop)
                    ups = list(si.on_update) if si.on_update else []
                    inst.sync_info = mybir.SyncInfo(on_wait=[waits[-1]], on_update=ups)
                out.append(inst)
            if changed:
                bb.instructions = out


def _build_bass(mode: str, reps: int):
    import concourse.bass as bass
    import concourse.mybir as mybir
    from concourse.tile import TileContext

    d_name, w_name, slab_mt, dma_chunks, stationary = _mode_cfg(mode)
    d_dt = getattr(mybir.dt, d_name)
    w_dt = getattr(mybir.dt, w_name)
    n_slabs = M_TILES // slab_mt

    nc = bass.Bass()
    dt_in = nc.declare_dram_parameter(
        "dt", [n_slabs, P, slab_mt, N_LOC], d_dt, isOutput=False
    )
    wt_in = nc.declare_dram_parameter("wt", [P, M_TILES, BO], w_dt, isOutput=False)
    if stationary:
        y_out = nc.declare_dram_parameter(
            "y", [P, 8, 2 * BO], mybir.dt.float32, isOutput=True
        )
    else:
        y_out = nc.declare_dram_parameter(
            "y", [BO, N_LOC], mybir.dt.float32, isOutput=True
        )

    dt_ap = dt_in.ap()

    def body(tc, pools):
        slab_pool, w_pool, ps_pool, out_pool = pools
        wt_sb = w_pool.tile([P, M_TILES, BO], w_dt, tag="wt")
        nc.sync.dma_start(wt_sb[:], wt_in.ap()[:])

        step = slab_mt // dma_chunks
        if stationary:
            # Spread the 16 nt accumulation regions over all 8 PSUM banks
            # (2 per bank) so back-to-back matmul drains alternate banks.
            ps = ps_pool.tile([P, 8, 512], mybir.dt.float32, tag="ps", name="ps")
            for jo in range(n_slabs):
                slab = slab_pool.tile([P, slab_mt, N_LOC], d_dt, tag="slab")
                for c in range(dma_chunks):
                    nc.sync.dma_start(
                        slab[:, c * step : (c + 1) * step, :],
                        dt_ap[jo][:, c * step : (c + 1) * step, :],
                    )
                for ji in range(slab_mt):
                    j = jo * slab_mt + ji
                    rhs = wt_sb[:, j, :]
                    for nt in range(NT):
                        # start=True clears has_written for the WHOLE target
                        # bank, so only the first matmul touching each bank
                        # (j==0, nt<8) may carry it; every region's first
                        # write then overwrites (bit clear) and later ones
                        # accumulate (bit set) — exactly the semantics needed.
                        bank, slot = nt % 8, nt // 8
                        nc.tensor.matmul(
                            ps[:, bank, slot * BO : (slot + 1) * BO],
                            slab[:, ji, nt * P : (nt + 1) * P],
                            rhs,
                            start=(j == 0 and nt < 8),
                            stop=(j == M_TILES - 1),
                            skip_group_check=True,
                        )
            out_sb = out_pool.tile([P, 8, 2 * BO], mybir.dt.float32, tag="out")
            nc.scalar.copy(out_sb[:], ps[:, :, 0 : 2 * BO])
            nc.sync.dma_start(y_out[:], out_sb[:])
        else:
            psums = [
                ps_pool.tile([BO, NB], mybir.dt.float32, tag=f"ps{nb}", name=f"ps{nb}")
                for nb in range(N_BLOCKS)
            ]
            for jo in range(n_slabs):
                slab = slab_pool.tile([P, slab_mt, N_LOC], d_dt, tag="slab")
                for c in range(dma_chunks):
                    nc.sync.dma_start(
                        slab[:, c * step : (c + 1) * step, :],
                        dt_ap[jo][:, c * step : (c + 1) * step, :],
                    )
                for ji in range(slab_mt):
                    j = jo * slab_mt + ji
                    lhsT = wt_sb[:, j, :]
                    for nb in range(N_BLOCKS):
                        rhs = slab[:, ji, nb * NB : (nb + 1) * NB]
                        nc.tensor.matmul(
                            psums[nb][:],
                            lhsT,
                            rhs,
                            start=(j == 0),
                            stop=(j == M_TILES - 1),
                        )
            out_sb = out_pool.tile([BO, N_LOC], mybir.dt.float32, tag="out")
            for nb in range(N_BLOCKS):
                nc.scalar.copy(out_sb[:, nb * NB : (nb + 1) * NB], psums[nb][:])
            nc.sync.dma_start(y_out[:], out_sb[:])

    with TileContext(nc) as tc:
        with (
            tc.tile_pool(name="slab", bufs=3) as slab_pool,
            tc.tile_pool(name="w", bufs=2) as w_pool,
            tc.tile_pool(name="psum", bufs=1, space="PSUM") as ps_pool,
            tc.tile_pool(name="out", bufs=1) as out_pool,
        ):
            pools = (slab_pool, w_pool, ps_pool, out_pool)
            if reps == 1:
                body(tc, pools)
            else:
                with tc.For_i(0, reps, 1):
                    body(tc, pools)

    _split_multi_waits(nc)
    return nc


class _Runner:
    """Compiled SPMD kernel with a reusable jitted callable."""

    def __init__(self, mode: str, reps: int):
        import jax
        from jax.sharding import Mesh, NamedSharding, PartitionSpec

        from jax.experimental.shard_map import shard_map
        import concourse.mybir as mybir
        from concourse.bass2jax import (
            _bass_exec_p,
            install_neuronx_cc_hook,
            partition_id_tensor,
        )

        self.jax = jax
        nc = _build_bass(mode, reps)
        install_neuronx_cc_hook()

        partition_name = (
            nc.partition_id_tensor.name if nc.partition_id_tensor else None
        )
        in_names, out_names, out_avals, self.zero_shapes = [], [], [], []
        for alloc in nc.m.functions[0].allocations:
            if not isinstance(alloc, mybir.MemoryLocationSet):
                continue
            name = alloc.memorylocations[0].name
            if alloc.kind == "ExternalInput":
                if name != partition_name:
                    in_names.append(name)
            elif alloc.kind == "ExternalOutput":
                out_names.append(name)
                shape = tuple(alloc.tensor_shape)
                np_dt = mybir.dt.np(alloc.dtype)
                out_avals.append(jax.core.ShapedArray(shape, np_dt))
                self.zero_shapes.append((shape, np_dt))
        n_params = len(in_names)
        n_outs = len(out_avals)
        in_names_all = in_names + out_names + (
            [partition_name] if partition_name else []
        )
        self.in_names = in_names
        self.out_names = out_names
        self.out_avals = out_avals

        def _bass_body(*args):
            operands = list(args)
            if partition_name is not None:
                operands.append(partition_id_tensor())
            outs = _bass_exec_p.bind(
                *operands,
                out_avals=tuple(out_avals),
                in_names=tuple(in_names_all),
                out_names=tuple(out_names),
                lowering_input_output_aliases=(),
                sim_require_finite=True,
                sim_require_nnan=True,
                nc=nc,
            )
            return tuple(outs)

        devices = jax.devices()[:N_CORES]
        assert len(devices) == N_CORES
        mesh = Mesh(np.asarray(devices), ("core",))
        self.sharding = NamedSharding(mesh, PartitionSpec("core"))
        self.fn = jax.jit(
            shard_map(
                _bass_body,
                mesh=mesh,
                in_specs=(PartitionSpec("core"),) * (n_params + n_outs),
                out_specs=(PartitionSpec("core"),) * n_outs,
                check_rep=False,
            ),
            donate_argnums=tuple(range(n_params, n_params + n_outs)),
            keep_unused=True,
        )

    def zeros(self):
        return [
            np.zeros((N_CORES * s[0], *s[1:]), d) for (s, d) in self.zero_shapes
        ]

    def __call__(self, concat_inputs):
        out = self.fn(*concat_inputs, *self.zeros())
        return [np.asarray(o) for o in out]


def _get_runner(mode: str, reps: int = 1) -> "_Runner":
    key = (mode, reps)
    if key not in _RUNNERS:
        _RUNNERS[key] = _Runner(mode, reps)
    return _RUNNERS[key]


def _np_dt(name):
    import concourse.mybir as mybir

    return np.dtype(mybir.dt.np(getattr(mybir.dt, name)))


def _prep_inputs(D, x, theta, mode=None):
    """Host-side shard prep: fold theta into x, quantize + pre-pack D slabs."""
    mode = mode or MODE
    d_name, w_name, slab_mt, _, _ = _mode_cfg(mode)
    d_np, w_np = _np_dt(d_name), _np_dt(w_name)
    n_slabs = M_TILES // slab_mt

    w = np.einsum("oc,bcm->bom", theta, x).reshape(BO, M).astype(np.float32)
    # [M, BO] -> [P, M_TILES, BO] with m = j*128 + p
    wt = np.ascontiguousarray(
        w.T.reshape(M_TILES, P, BO).transpose(1, 0, 2)
    ).astype(w_np)
    wt_cat = np.ascontiguousarray(np.tile(wt, (N_CORES, 1, 1)))

    Dq = np.ascontiguousarray(D).astype(d_np)
    # D[n, m]; n = c*N_LOC + nl; m = jo*(slab_mt*P) + ji*P + p
    # target per core: [jo, p, ji, nl]
    dt = Dq.reshape(N_CORES, N_LOC, n_slabs, slab_mt, P).transpose(0, 2, 4, 3, 1)
    dt_cat = np.ascontiguousarray(dt).reshape(N_CORES * n_slabs, P, slab_mt, N_LOC)
    return {"dt": dt_cat, "wt": wt_cat}


def kernel(D, x, theta, bias):
    D = np.asarray(D, dtype=np.float32)
    x = np.asarray(x, dtype=np.float32)
    theta = np.asarray(theta, dtype=np.float32)
    bias = np.asarray(bias, dtype=np.float32)

    stationary = _mode_cfg(MODE)[4]
    runner = _get_runner(MODE, 1)
    inputs = _prep_inputs(D, x, theta, MODE)
    concat = [inputs[name] for name in runner.in_names]
    outs = runner(concat)
    y_cat = outs[runner.out_names.index("y")]
    y = np.empty((B, C_OUT, N), dtype=np.float32)
    if stationary:
        # y_cat: [8*P, 8, 2*BO] -> per core [p, bank, slot, bo];
        # nt = slot*8 + bank, n = c*2048 + nt*128 + p
        yc = y_cat.reshape(N_CORES, P, 8, 2, BO).transpose(0, 3, 2, 1, 4)
        yc = yc.reshape(N_CORES * N_LOC, BO).T.reshape(B, C_OUT, N)
        y[:] = yc
    else:
        for c in range(N_CORES):
            yc = y_cat[c * BO : (c + 1) * BO]  # [16, N_LOC]
            y[:, :, c * N_LOC : (c + 1) * N_LOC] = yc.reshape(B, C_OUT, N_LOC)
    return y + bias
